# revision 13
# baseline (speedup 1.0000x reference)
"""DGCNN classification forward pass on 8 Trainium2 NeuronCores.

Strategy: data-parallel over batch B=8 (one point cloud per core); all
weights replicated.  Per core, each EdgeConv layer is reformulated as:

    h[n,k,o] = LeakyReLU(s_o * (W @ [nbr-ctr; ctr]) + b_o)
             = LeakyReLU(U[o, idx(n,k)] + V[o, n])
  with U = (s . Wn)^T p   (per point),  V = (s . (Wc-Wn))^T p + b
    out[n,o] = max_k h[n,k,o] = LeakyReLU(max_k U[o, idx(n,k)] + V[o,n])

(valid because s_o > 0 and LeakyReLU is monotone).  So each layer is:
  1) U^T, V^T via two small matmuls (PE), U^T staged to DRAM
  2) exact KNN: d-tile matmuls (PE) -> top-32 per row via DVE
     max8 / max_index / match_replace rounds
  3) neighbor gather of U^T rows via SWDGE dma_gather
  4) segmented max over k (DVE reduce), +V, LeakyReLU
The final MLP head runs in [feature, point] layout so folded-BN scales and
biases are per-partition scalars.
"""

import os
import sys
import numpy as np

sys.path.insert(0, "/opt/trn_rl_repo")
sys.path.insert(0, "/opt/trn_rl_repo/concourse")

import concourse.bass as bass
import concourse.bacc as bacc
import concourse.mybir as mybir
from concourse import tile
from concourse.bass_utils import run_bass_kernel_spmd

F32 = mybir.dt.float32
U16 = mybir.dt.uint16
I16 = mybir.dt.int16

N = 2048
K = 32
NT = N // 128  # 16 point tiles
NEG = 0.2
NEG_INF = -3.0e38

# layer configs: (c_in, c_out)
LAYERS = [(3, 64), (64, 64), (64, 128), (128, 256)]

Alu = mybir.AluOpType
Act = mybir.ActivationFunctionType
Axis = mybir.AxisListType


def _leaky(nc, out, in_, accum_out=None):
    # out = max(in_ * NEG, in_) == LeakyReLU(in_, NEG)  (one DVE op)
    nc.vector.scalar_tensor_tensor(
        out, in_, NEG, in_, Alu.mult, Alu.max, accum_out=accum_out
    )


def build_module():
    nc = bacc.Bacc(
        "TRN2",
        target_bir_lowering=False,
        debug=False,
        enable_asserts=False,
        num_devices=8,
        num_swdge_queues=4,
    )

    # ---- external tensors ------------------------------------------------
    def din(name, shape):
        return nc.dram_tensor(name, list(shape), F32, kind="ExternalInput").ap()

    t_x = din("xb", (3, N))
    conv_w = []
    for li, (c, o) in enumerate(LAYERS, start=1):
        conv_w.append(
            (
                din(f"A{li}", (c, o)),
                din(f"B{li}", (c, o)),
                din(f"br{li}", (1, o)),
            )
        )
    t_A5 = [
        din("A51", (64, 1024)),
        din("A52", (64, 1024)),
        din("A53", (128, 1024)),
        din("A54a", (128, 1024)),
        din("A54b", (128, 1024)),
    ]
    t_b5 = din("b5v", (1024,))
    t_L1A = din("L1A", (2048, 512))
    t_b6 = din("b6r", (1, 512))
    t_L2A = din("L2A", (512, 256))
    t_L2b = din("L2br", (1, 256))
    t_L3A = din("L3A", (256, 5))
    t_L3b = din("L3br", (1, 5))
    t_F1A = din("F1A", (1024, 512))
    t_b8 = din("b8r", (1, 512))
    t_F2A = din("F2A", (512, 256))
    t_F2b = din("F2br", (1, 256))
    t_F3A = din("F3A", (256, 5))
    t_F3b = din("F3br", (1, 5))
    t_ident = din("ident", (128, 128))
    t_onesrow = din("onesrow", (1, 128))
    t_onescol = din("onescol", (128, 1))

    t_go = nc.dram_tensor("go", [5, 1], F32, kind="ExternalOutput").ap()
    t_yo = nc.dram_tensor("yo", [5, 1], F32, kind="ExternalOutput").ap()

    t_dbg = None
    if os.environ.get("KERNEL_DEBUG"):
        t_dbg = {
            "dF1": nc.dram_tensor("dF1", [64, N], F32, kind="ExternalOutput").ap(),
            "dF2": nc.dram_tensor("dF2", [64, N], F32, kind="ExternalOutput").ap(),
            "dF3": nc.dram_tensor("dF3", [128, N], F32, kind="ExternalOutput").ap(),
            "dF4a": nc.dram_tensor("dF4a", [128, N], F32, kind="ExternalOutput").ap(),
            "dF4b": nc.dram_tensor("dF4b", [128, N], F32, kind="ExternalOutput").ap(),
            "dmaxh": nc.dram_tensor("dmaxh", [128, 8], F32, kind="ExternalOutput").ap(),
            "dsumh": nc.dram_tensor("dsumh", [128, 8], F32, kind="ExternalOutput").ap(),
            "dgidx1": nc.dram_tensor("dgidx1", [128, 16 * K], F32, kind="ExternalOutput").ap(),
            "dnbr1": nc.dram_tensor("dnbr1", [128, K * 64], F32, kind="ExternalOutput").ap(),
        }

    with tile.TileContext(nc) as tc:
        build_kernel(
            nc, tc,
            t_x, conv_w, t_A5, t_b5,
            t_L1A, t_b6, t_L2A, t_L2b, t_L3A, t_L3b,
            t_F1A, t_b8, t_F2A, t_F2b, t_F3A, t_F3b,
            t_ident, t_onesrow, t_onescol, t_go, t_yo, t_dbg,
        )

    nc.compile()
    return nc


def build_kernel(
    nc, tc,
    t_x, conv_w, t_A5, t_b5,
    t_L1A, t_b6, t_L2A, t_L2b, t_L3A, t_L3b,
    t_F1A, t_b8, t_F2A, t_F2b, t_F3A, t_F3b,
    t_ident, t_onesrow, t_onescol, t_go, t_yo, t_dbg=None,
):
    from contextlib import ExitStack

    reps = int(os.environ.get("KERNEL_REPS", "1"))
    for _rep in range(reps):
        _build_once(
            nc, tc,
            t_x, conv_w, t_A5, t_b5,
            t_L1A, t_b6, t_L2A, t_L2b, t_L3A, t_L3b,
            t_F1A, t_b8, t_F2A, t_F2b, t_F3A, t_F3b,
            t_ident, t_onesrow, t_onescol, t_go, t_yo,
            t_dbg if _rep == reps - 1 else None,
        )


def _build_once(
    nc, tc,
    t_x, conv_w, t_A5, t_b5,
    t_L1A, t_b6, t_L2A, t_L2b, t_L3A, t_L3b,
    t_F1A, t_b8, t_F2A, t_F2b, t_F3A, t_F3b,
    t_ident, t_onesrow, t_onescol, t_go, t_yo, t_dbg=None,
):
    from contextlib import ExitStack

    ctx = ExitStack()
    with ctx:
        const = ctx.enter_context(tc.tile_pool(name="const", bufs=1))
        feat = ctx.enter_context(tc.tile_pool(name="feat", bufs=1))
        dram = ctx.enter_context(tc.tile_pool(name="dram", bufs=1, space="DRAM"))

        from concourse import library_config
        nc.gpsimd.load_library(library_config.mlp)

        # constants
        ident = const.tile([128, 128], F32, tag="ident")
        nc.sync.dma_start(ident, t_ident)
        onesrow = const.tile([1, 128], F32, tag="onesrow")
        nc.sync.dma_start(onesrow, t_onesrow)
        onescol = const.tile([128, 1], F32, tag="onescol")
        nc.sync.dma_start(onescol, t_onescol)

        # persistent feature maps (feature-major [c, N])
        F0 = feat.tile([3, N], F32, tag="F0")
        nc.sync.dma_start(F0, t_x)
        F1 = feat.tile([64, N], F32, tag="F1")
        F2 = feat.tile([64, N], F32, tag="F2")
        F3 = feat.tile([128, N], F32, tag="F3")
        F4a = feat.tile([128, N], F32, tag="F4a")
        F4b = feat.tile([128, N], F32, tag="F4b")

        fins = [F0, F1, F2, F3]
        fouts = [[F1], [F2], [F3], [F4a, F4b]]

        # layer weights in SBUF
        wsb = []
        for li, ((c, o), (tA, tB, tbr)) in enumerate(zip(LAYERS, conv_w), start=1):
            Asb = const.tile([c, o], F32, tag=f"A{li}", name=f"A{li}sb")
            nc.sync.dma_start(Asb, tA)
            Bsb = const.tile([c, o], F32, tag=f"B{li}", name=f"B{li}sb")
            nc.sync.dma_start(Bsb, tB)
            brsb = const.tile([1, o], F32, tag=f"br{li}", name=f"br{li}sb")
            nc.sync.dma_start(brsb, tbr)
            wsb.append((Asb, Bsb, brsb))

        # ------------------- EdgeConv layers -------------------
        lw = ctx.enter_context(tc.tile_pool(name="lw", bufs=1))
        luv = ctx.enter_context(tc.tile_pool(name="luv", bufs=3))
        lk = ctx.enter_context(tc.tile_pool(name="lk", bufs=3))
        lkd = ctx.enter_context(tc.tile_pool(name="lkd", bufs=4, space="DRAM"))
        psUV = ctx.enter_context(tc.tile_pool(name="psUV", bufs=2, space="PSUM"))
        psD = ctx.enter_context(tc.tile_pool(name="psD", bufs=2, space="PSUM"))
        psT = ctx.enter_context(tc.tile_pool(name="psT", bufs=2, space="PSUM"))
        for li, ((c, o), (Asb, Bsb, brsb)) in enumerate(zip(LAYERS, wsb), start=1):
            Fin = fins[li - 1]
            Fout = fouts[li - 1]
            UTd = dram.tile([N, o], F32, tag=f"UT{li}", name=f"UT{li}d")

            if True:
                if True:
                    # prep: P2 = 2*Fin (ACT), PP = Fin^2 (ACT), negxx
                    P2 = lw.tile([c, N], F32, tag="p2", name="p2")
                    nc.scalar.activation(P2, Fin, Act.Copy, bias=0.0, scale=2.0)
                    PP = lw.tile([c, N], F32, tag="pp", name="pp")
                    nc.scalar.activation(PP, Fin, Act.Square)
                    negxx = lw.tile([1, N], F32, tag="negxx", name="negxx")
                    for ch in range(4):
                        xps = psUV.tile([1, 512], F32, tag="uv", name="xps")
                        nc.tensor.matmul(
                            xps, onescol[0:c, :], PP[:, ch * 512:(ch + 1) * 512],
                            start=True, stop=True,
                        )
                        nc.scalar.activation(
                            negxx[:, ch * 512:(ch + 1) * 512], xps, Act.Copy,
                            bias=0.0, scale=-1.0,
                        )

                    # U^T -> DRAM;  V^T -> SBUF
                    Vt = lw.tile([128, NT * o], F32, tag="vt", name="vt")
                    for t in range(NT):
                        sl = slice(t * 128, (t + 1) * 128)
                        ups = psUV.tile([128, o], F32, tag="uv", name="ups")
                        nc.tensor.matmul(ups, Fin[:, sl], Asb, start=True, stop=True)
                        usb = luv.tile([128, o], F32, tag="usb", name="usb")
                        nc.scalar.activation(usb, ups, Act.Copy)
                        nc.sync.dma_start(UTd[sl, :], usb)
                        vps = psUV.tile([128, o], F32, tag="uv", name="vps")
                        nc.tensor.matmul(vps, Fin[:, sl], Bsb, start=True, stop=False)
                        nc.tensor.matmul(vps, onesrow, brsb, start=False, stop=True)
                        nc.scalar.activation(
                            Vt[:, t * o:(t + 1) * o], vps, Act.Copy
                        )

                # KNN + gather + max per point tile
                if True:
                    for t in range(NT):
                        sl = slice(t * 128, (t + 1) * 128)
                        dds = lk.tile([128, N], F32, tag="dds", name="dds")
                        for hh in range(2):
                            ddp = psD.tile([128, N // 2], F32, tag="dd",
                                           name="ddp")
                            for ch in range(2):
                                cs = slice(hh * 1024 + ch * 512,
                                           hh * 1024 + (ch + 1) * 512)
                                cp = slice(ch * 512, (ch + 1) * 512)
                                nc.tensor.matmul(
                                    ddp[:, cp], P2[:, sl], Fin[:, cs],
                                    start=True, stop=False,
                                )
                                nc.tensor.matmul(
                                    ddp[:, cp], onesrow, negxx[:, cs],
                                    start=False, stop=True,
                                )
                            nc.scalar.activation(
                                dds[:, hh * 1024:(hh + 1) * 1024], ddp,
                                Act.Copy)

                        gidx = lk.tile([128, K], U16, tag="gidx", name="gidx")
                        for r in range(4):
                            v8 = lk.tile([128, 8], F32, tag="v8", name="v8")
                            nc.vector.max(v8, dds)
                            nc.vector.max_index(gidx[:, r * 8:(r + 1) * 8], v8, dds)
                            if r < 3:
                                nc.vector.match_replace(dds, v8, dds, NEG_INF)

                        if t_dbg is not None and li == 1:
                            gidxf = lk.tile([128, K], F32, tag="gidxf",
                                            name="gidxf")
                            nc.vector.tensor_copy(gidxf, gidx)
                            nc.sync.dma_start(
                                t_dbg["dgidx1"][:, t * K:(t + 1) * K], gidxf
                            )
                        # SWDGE wrapped index layout: list[i] lives at
                        # storage[i % 16, i // 16]; we need
                        # list[k*128 + p] = gidx[p, k]  =>
                        # widx[q, 8k+u] = gidx[16u+q, k]
                        ld = lkd.tile([128, K], I16, tag="ld", name="ld")
                        nc.sync.dma_start(ld, gidx.bitcast(I16))
                        widx = lk.tile([128, 256], I16, tag="widx", name="widx")
                        ldw = ld.rearrange("(u q) k -> q k u", u=8)
                        for g in range(8):
                            nc.sync.dma_start(
                                widx[g * 16:(g + 1) * 16, :].rearrange(
                                    "q (k u) -> q k u", u=8
                                ),
                                ldw,
                            )

                        nbr = lk.tile([128, K * o], F32, tag="nbr", name="nbr")
                        nbrv = nbr.rearrange("p (k o) -> p k o", k=K)
                        for gc in range(4):
                            nc.gpsimd.dma_gather(
                                nbrv[:, gc * 8:(gc + 1) * 8, :],
                                UTd,
                                widx[:, gc * 64:(gc + 1) * 64],
                                1024,
                                1024,
                                o,
                                queue_num=(t * 4 + gc) % 4,
                            )

                        if t_dbg is not None and li == 1 and t == 0:
                            nc.sync.dma_start(t_dbg["dnbr1"], nbr)
                        M = lk.tile([128, o], F32, tag="m", name="mtile")
                        nc.vector.tensor_reduce(
                            M, nbr.rearrange("p (k o) -> p o k", k=K),
                            Axis.X, Alu.max,
                        )
                        nc.vector.tensor_tensor(
                            M, M, Vt[:, t * o:(t + 1) * o], Alu.add
                        )
                        act = lk.tile([128, o], F32, tag="act", name="acttile")
                        _leaky(nc, act, M)

                        # transpose back to feature-major
                        for bi, Fo in enumerate(Fout):
                            bw = min(128, o - bi * 128)
                            tp = psT.tile([128, 128], F32, tag="tp", name="tp")
                            nc.tensor.transpose(
                                tp[0:bw, :], act[:, bi * 128:bi * 128 + bw], ident
                            )
                            nc.scalar.activation(Fo[0:bw, sl], tp[0:bw, :], Act.Copy)

        if t_dbg is not None:
            nc.sync.dma_start(t_dbg["dF1"], F1)
            nc.sync.dma_start(t_dbg["dF2"], F2)
            nc.sync.dma_start(t_dbg["dF3"], F3)
            nc.sync.dma_start(t_dbg["dF4a"], F4a)
            nc.sync.dma_start(t_dbg["dF4b"], F4b)

        # ------------------- global feature + heads -------------------
        with tc.tile_pool(name="fcw", bufs=1) as fcw, \
             tc.tile_pool(name="fcwk", bufs=2) as fcwk, \
             tc.tile_pool(name="psh", bufs=2, space="PSUM") as psH, \
             tc.tile_pool(name="psf", bufs=4, space="PSUM") as psF:
            A5sb = []
            for i, tA in enumerate(t_A5):
                p = 64 if i < 2 else 128
                a = fcw.tile([p, 1024], F32, tag=f"A5_{i}", name=f"A5sb{i}")
                nc.sync.dma_start(a, tA)
                A5sb.append(a)
            b5t = fcw.tile([128, 8], F32, tag="b5t")
            nc.sync.dma_start(b5t, t_b5.rearrange("(i p) -> p i", p=128))

            L1Asb = fcw.tile([128, 16 * 512], F32, tag="L1A", name="L1Asb")
            nc.sync.dma_start(
                L1Asb.rearrange("p (j o) -> p j o", j=16),
                t_L1A.rearrange("(j p) o -> p j o", p=128),
            )
            F1Asb = fcw.tile([128, 8 * 512], F32, tag="F1A", name="F1Asb")
            nc.sync.dma_start(
                F1Asb.rearrange("p (j o) -> p j o", j=8),
                t_F1A.rearrange("(j p) o -> p j o", p=128),
            )
            L2Asb = fcw.tile([128, 4 * 256], F32, tag="L2A", name="L2Asb")
            nc.sync.dma_start(
                L2Asb.rearrange("p (j o) -> p j o", j=4),
                t_L2A.rearrange("(j p) o -> p j o", p=128),
            )
            F2Asb = fcw.tile([128, 4 * 256], F32, tag="F2A", name="F2Asb")
            nc.sync.dma_start(
                F2Asb.rearrange("p (j o) -> p j o", j=4),
                t_F2A.rearrange("(j p) o -> p j o", p=128),
            )
            L3Asb = fcw.tile([128, 2 * 5], F32, tag="L3A", name="L3Asb")
            nc.sync.dma_start(
                L3Asb.rearrange("p (j o) -> p j o", j=2),
                t_L3A.rearrange("(j p) o -> p j o", p=128),
            )
            F3Asb = fcw.tile([128, 2 * 5], F32, tag="F3A", name="F3Asb")
            nc.sync.dma_start(
                F3Asb.rearrange("p (j o) -> p j o", j=2),
                t_F3A.rearrange("(j p) o -> p j o", p=128),
            )
            b6sb = fcw.tile([1, 512], F32, tag="b6")
            nc.sync.dma_start(b6sb, t_b6)
            b8sb = fcw.tile([1, 512], F32, tag="b8")
            nc.sync.dma_start(b8sb, t_b8)
            L2bsb = fcw.tile([1, 256], F32, tag="L2b")
            nc.sync.dma_start(L2bsb, t_L2b)
            F2bsb = fcw.tile([1, 256], F32, tag="F2b")
            nc.sync.dma_start(F2bsb, t_F2b)
            L3bsb = fcw.tile([1, 5], F32, tag="L3b")
            nc.sync.dma_start(L3bsb, t_L3b)
            F3bsb = fcw.tile([1, 5], F32, tag="F3b")
            nc.sync.dma_start(F3bsb, t_F3b)

            # h = LeakyReLU(A5 @ cat + b5) in [o, n] layout; pool max+sum
            pieces = [
                (A5sb[0], F1, 64),
                (A5sb[1], F2, 64),
                (A5sb[2], F3, 128),
                (A5sb[3], F4a, 128),
                (A5sb[4], F4b, 128),
            ]
            maxh = fcw.tile([128, 8], F32, tag="maxh")
            sumh = fcw.tile([128, 8], F32, tag="sumh")
            for ot in range(8):
                osl = slice(ot * 128, (ot + 1) * 128)
                cmax = fcwk.tile([128, 4], F32, tag="cmax", name="cmax")
                csum = fcwk.tile([128, 4], F32, tag="csum", name="csum")
                for nch in range(4):
                    nsl = slice(nch * 512, (nch + 1) * 512)
                    hps = psH.tile([128, 512], F32, tag="h", name="hps")
                    for i, (Ax, Fx, kk) in enumerate(pieces):
                        nc.tensor.matmul(
                            hps, Ax[:, osl], Fx[:, nsl],
                            start=(i == 0), stop=(i == len(pieces) - 1),
                        )
                    ht = fcwk.tile([128, 512], F32, tag="ht", name="ht")
                    # ht = h + b5, then leaky with sum accum (DVE)
                    nc.vector.tensor_scalar(
                        ht, hps, b5t[:, ot:ot + 1], None, Alu.add
                    )
                    hl = fcwk.tile([128, 512], F32, tag="hl", name="hl")
                    _leaky(nc, hl, ht, accum_out=csum[:, nch:nch + 1])
                    nc.vector.tensor_reduce(
                        cmax[:, nch:nch + 1], hl, Axis.X, Alu.max
                    )
                nc.vector.tensor_reduce(
                    maxh[:, ot:ot + 1], cmax, Axis.X, Alu.max
                )
                nc.vector.tensor_reduce(
                    sumh[:, ot:ot + 1], csum, Axis.X, Alu.add
                )

            if t_dbg is not None:
                nc.sync.dma_start(t_dbg["dmaxh"], maxh)
                nc.sync.dma_start(t_dbg["dsumh"], sumh)

            def fc(lhs_sb, nj, rhs_cols, bias_sb, width, out_cols, act_fn=True):
                """out[width] = (LeakyReLU?)(lhsT.T @ rhs + bias). Returns
                [128, ceil(width/128)] tile whose columns are 128-chunks."""
                nm = (width + 127) // 128
                res = fcwk.tile([128, max(nm, 1)], F32, tag=f"fc{width}_{nj}",
                                name="fcres")
                for m in range(nm):
                    mw = min(128, width - m * 128)
                    zps = psF.tile([128, 1], F32, tag="z", name="zps")
                    for j in range(nj):
                        nc.tensor.matmul(
                            zps[0:mw, :],
                            lhs_sb.rearrange("p (j o) -> p j o", j=nj)[
                                :, j, m * 128:m * 128 + mw
                            ],
                            rhs_cols[j],
                            start=(j == 0), stop=False,
                        )
                    nc.tensor.matmul(
                        zps[0:mw, :],
                        bias_sb[:, m * 128:m * 128 + mw],
                        onesrow[:, 0:1],
                        start=False, stop=True,
                    )
                    nc.scalar.activation(
                        res[0:mw, m:m + 1], zps[0:mw, :], Act.Copy
                    )
                if act_fn:
                    _leaky(nc, res, res)
                return res

            # x-branch (g): L1 (K=2048: max||sum), L2, L3
            g_rhs = [maxh[:, j:j + 1] for j in range(8)] + \
                    [sumh[:, j:j + 1] for j in range(8)]
            z1 = fc(L1Asb, 16, g_rhs, b6sb, 512, 4)
            z1_rhs = [z1[:, j:j + 1] for j in range(4)]
            z2 = fc(L2Asb, 4, z1_rhs, L2bsb, 256, 2)
            z2_rhs = [z2[:, j:j + 1] for j in range(2)]
            z3 = fc(L3Asb, 2, z2_rhs, L3bsb, 5, 1, act_fn=False)
            nc.sync.dma_start(t_go, z3[0:5, 0:1])

            # y-branch: F1 (K=1024: max only), F2, F3
            y_rhs = [maxh[:, j:j + 1] for j in range(8)]
            w1 = fc(F1Asb, 8, y_rhs, b8sb, 512, 4)
            w1_rhs = [w1[:, j:j + 1] for j in range(4)]
            w2 = fc(F2Asb, 4, w1_rhs, F2bsb, 256, 2)
            w2_rhs = [w2[:, j:j + 1] for j in range(2)]
            w3 = fc(F3Asb, 2, w2_rhs, F3bsb, 5, 1, act_fn=False)
            nc.sync.dma_start(t_yo, w3[0:5, 0:1])


# --------------------------------------------------------------------------
# host side
# --------------------------------------------------------------------------

_NC = None


def _get_nc():
    global _NC
    if _NC is None:
        _NC = build_module()
    return _NC


def _prep_weights(inp):
    f = lambda k: np.ascontiguousarray(np.asarray(inp[k], dtype=np.float32))
    d = {}

    for li, (c, o) in enumerate(LAYERS, start=1):
        W = f(f"W{li}")          # [o, 2c]
        s = f(f"s{li}")          # [o]
        b = f(f"b{li}")          # [o]
        Wn = W[:, :c]
        Wc = W[:, c:]
        d[f"A{li}"] = np.ascontiguousarray((s[:, None] * Wn).T)
        d[f"B{li}"] = np.ascontiguousarray((s[:, None] * (Wc - Wn)).T)
        d[f"br{li}"] = b[None, :].copy()

    A5 = np.ascontiguousarray((f("s5")[:, None] * f("W5")).T)   # [512, 1024]
    d["A51"] = A5[0:64].copy()
    d["A52"] = A5[64:128].copy()
    d["A53"] = A5[128:256].copy()
    d["A54a"] = A5[256:384].copy()
    d["A54b"] = A5[384:512].copy()
    d["b5v"] = f("b5")

    L1 = (f("s6")[:, None] * f("L1w")).T.copy()                 # [2048, 512]
    L1[1024:] /= float(N)
    d["L1A"] = np.ascontiguousarray(L1)
    d["b6r"] = f("b6")[None, :].copy()
    d["L2A"] = np.ascontiguousarray((f("s7")[:, None] * f("L2w")).T)
    d["L2br"] = (f("s7") * f("L2b") + f("b7"))[None, :].copy()
    d["L3A"] = np.ascontiguousarray(f("L3w").T)
    d["L3br"] = f("L3b")[None, :].copy()

    d["F1A"] = np.ascontiguousarray((f("s8")[:, None] * f("F1w")).T)
    d["b8r"] = f("b8")[None, :].copy()
    d["F2A"] = np.ascontiguousarray((f("s9")[:, None] * f("F2w")).T)
    d["F2br"] = (f("s9") * f("F2b") + f("b9"))[None, :].copy()
    d["F3A"] = np.ascontiguousarray(f("F3w").T)
    d["F3br"] = f("F3b")[None, :].copy()

    d["ident"] = np.eye(128, dtype=np.float32)
    d["onesrow"] = np.ones((1, 128), dtype=np.float32)
    d["onescol"] = np.ones((128, 1), dtype=np.float32)
    return d


def kernel(**inputs):
    x = np.asarray(inputs["x"], dtype=np.float32)   # [8, 3, N]
    B = x.shape[0]
    assert B == 8 and x.shape[1] == 3 and x.shape[2] == N

    shared = _prep_weights(inputs)
    in_maps = []
    for bidx in range(B):
        m = dict(shared)
        m["xb"] = np.ascontiguousarray(x[bidx])
        in_maps.append(m)

    nc = _get_nc()
    res = run_bass_kernel_spmd(nc, in_maps, core_ids=list(range(B)))
    g = np.stack([res.results[i]["go"].reshape(5) for i in range(B)])
    y = np.stack([res.results[i]["yo"].reshape(5) for i in range(B)])
    return (g.astype(np.float32), y.astype(np.float32))


if __name__ == "__main__":
    # smoke test with random data
    rng = np.random.default_rng(0)
    print("building module...")
    nc = _get_nc()
    print("built ok")


# revision 18
# speedup vs baseline: 1.7956x; 1.7956x over previous
"""DGCNN classification forward pass on 8 Trainium2 NeuronCores.

Strategy: data-parallel over batch B=8 (one point cloud per core); all
weights replicated.  Per core, each EdgeConv layer is reformulated as:

    h[n,k,o] = LeakyReLU(s_o * (W @ [nbr-ctr; ctr]) + b_o)
             = LeakyReLU(U[o, idx(n,k)] + V[o, n])
  with U = (s . Wn)^T p   (per point),  V = (s . (Wc-Wn))^T p + b
    out[n,o] = max_k h[n,k,o] = LeakyReLU(max_k U[o, idx(n,k)] + V[o,n])

(valid because s_o > 0 and LeakyReLU is monotone).  So each layer is:
  1) U^T, V^T via two small matmuls (PE), U^T staged to DRAM
  2) exact KNN: d-tile matmuls (PE) -> top-32 per row via DVE
     max8 / max_index / match_replace rounds
  3) neighbor gather of U^T rows via SWDGE dma_gather
  4) segmented max over k (DVE reduce), +V, LeakyReLU
The final MLP head runs in [feature, point] layout so folded-BN scales and
biases are per-partition scalars.
"""

import os
import sys
import numpy as np

sys.path.insert(0, "/opt/trn_rl_repo")
sys.path.insert(0, "/opt/trn_rl_repo/concourse")

import concourse.bass as bass
import concourse.bacc as bacc
import concourse.mybir as mybir
from concourse import tile
from concourse.bass_utils import run_bass_kernel_spmd

F32 = mybir.dt.float32
U16 = mybir.dt.uint16
I16 = mybir.dt.int16
F16 = mybir.dt.float16

N = 2048
K = 32
NT = N // 128  # 16 point tiles
NEG = 0.2
NEG_INF = -3.0e38

# layer configs: (c_in, c_out)
LAYERS = [(3, 64), (64, 64), (64, 128), (128, 256)]

Alu = mybir.AluOpType
Act = mybir.ActivationFunctionType
Axis = mybir.AxisListType


def _leaky(nc, out, in_, accum_out=None):
    # out = max(in_ * NEG, in_) == LeakyReLU(in_, NEG)  (one DVE op)
    nc.vector.scalar_tensor_tensor(
        out, in_, NEG, in_, Alu.mult, Alu.max, accum_out=accum_out
    )


def build_module():
    nc = bacc.Bacc(
        "TRN2",
        target_bir_lowering=False,
        debug=False,
        enable_asserts=False,
        num_devices=8,
        num_swdge_queues=4,
    )

    # ---- external tensors ------------------------------------------------
    def din(name, shape):
        return nc.dram_tensor(name, list(shape), F32, kind="ExternalInput").ap()

    t_x = din("xb", (3, N))
    conv_w = []
    for li, (c, o) in enumerate(LAYERS, start=1):
        conv_w.append(
            (
                din(f"A{li}", (c, o)),
                din(f"B{li}", (c, o)),
                din(f"br{li}", (1, o)),
            )
        )
    t_A5 = [
        din("A51", (64, 1024)),
        din("A52", (64, 1024)),
        din("A53", (128, 1024)),
        din("A54a", (128, 1024)),
        din("A54b", (128, 1024)),
    ]
    t_b5 = din("b5v", (1024,))
    t_L1A = din("L1A", (2048, 512))
    t_b6 = din("b6r", (1, 512))
    t_L2A = din("L2A", (512, 256))
    t_L2b = din("L2br", (1, 256))
    t_L3A = din("L3A", (256, 5))
    t_L3b = din("L3br", (1, 5))
    t_F1A = din("F1A", (1024, 512))
    t_b8 = din("b8r", (1, 512))
    t_F2A = din("F2A", (512, 256))
    t_F2b = din("F2br", (1, 256))
    t_F3A = din("F3A", (256, 5))
    t_F3b = din("F3br", (1, 5))
    t_ident = din("ident", (128, 128))
    t_onesrow = din("onesrow", (1, 128))
    t_onescol = din("onescol", (128, 1))

    t_go = nc.dram_tensor("go", [5, 1], F32, kind="ExternalOutput").ap()
    t_yo = nc.dram_tensor("yo", [5, 1], F32, kind="ExternalOutput").ap()

    t_dbg = None
    if os.environ.get("KERNEL_DEBUG"):
        t_dbg = {
            "dF1": nc.dram_tensor("dF1", [64, N], F32, kind="ExternalOutput").ap(),
            "dF2": nc.dram_tensor("dF2", [64, N], F32, kind="ExternalOutput").ap(),
            "dF3": nc.dram_tensor("dF3", [128, N], F32, kind="ExternalOutput").ap(),
            "dF4a": nc.dram_tensor("dF4a", [128, N], F32, kind="ExternalOutput").ap(),
            "dF4b": nc.dram_tensor("dF4b", [128, N], F32, kind="ExternalOutput").ap(),
            "dmaxh": nc.dram_tensor("dmaxh", [128, 8], F32, kind="ExternalOutput").ap(),
            "dsumh": nc.dram_tensor("dsumh", [128, 8], F32, kind="ExternalOutput").ap(),
            "dgidx1": nc.dram_tensor("dgidx1", [128, 16 * K], F32, kind="ExternalOutput").ap(),
            "dnbr1": nc.dram_tensor("dnbr1", [128, K * 64], F32, kind="ExternalOutput").ap(),
        }

    with tile.TileContext(nc) as tc:
        build_kernel(
            nc, tc,
            t_x, conv_w, t_A5, t_b5,
            t_L1A, t_b6, t_L2A, t_L2b, t_L3A, t_L3b,
            t_F1A, t_b8, t_F2A, t_F2b, t_F3A, t_F3b,
            t_ident, t_onesrow, t_onescol, t_go, t_yo, t_dbg,
        )

    nc.compile()
    return nc


def build_kernel(
    nc, tc,
    t_x, conv_w, t_A5, t_b5,
    t_L1A, t_b6, t_L2A, t_L2b, t_L3A, t_L3b,
    t_F1A, t_b8, t_F2A, t_F2b, t_F3A, t_F3b,
    t_ident, t_onesrow, t_onescol, t_go, t_yo, t_dbg=None,
):
    from contextlib import ExitStack

    reps = int(os.environ.get("KERNEL_REPS", "1"))
    for _rep in range(reps):
        _build_once(
            nc, tc,
            t_x, conv_w, t_A5, t_b5,
            t_L1A, t_b6, t_L2A, t_L2b, t_L3A, t_L3b,
            t_F1A, t_b8, t_F2A, t_F2b, t_F3A, t_F3b,
            t_ident, t_onesrow, t_onescol, t_go, t_yo,
            t_dbg if _rep == reps - 1 else None,
        )


def _build_once(
    nc, tc,
    t_x, conv_w, t_A5, t_b5,
    t_L1A, t_b6, t_L2A, t_L2b, t_L3A, t_L3b,
    t_F1A, t_b8, t_F2A, t_F2b, t_F3A, t_F3b,
    t_ident, t_onesrow, t_onescol, t_go, t_yo, t_dbg=None,
):
    from contextlib import ExitStack

    ctx = ExitStack()
    with ctx:
        const = ctx.enter_context(tc.tile_pool(name="const", bufs=1))
        feat = ctx.enter_context(tc.tile_pool(name="feat", bufs=1))
        dram = ctx.enter_context(tc.tile_pool(name="dram", bufs=1, space="DRAM"))

        from concourse import library_config
        nc.gpsimd.load_library(library_config.mlp)

        # constants
        ident = const.tile([128, 128], F32, tag="ident")
        nc.sync.dma_start(ident, t_ident)
        onesrow = const.tile([1, 128], F32, tag="onesrow")
        nc.sync.dma_start(onesrow, t_onesrow)
        onescol = const.tile([128, 1], F32, tag="onescol")
        nc.sync.dma_start(onescol, t_onescol)

        # persistent feature maps (feature-major [c, N])
        F0 = feat.tile([3, N], F32, tag="F0")
        nc.sync.dma_start(F0, t_x)
        F1 = feat.tile([64, N], F32, tag="F1")
        F2 = feat.tile([64, N], F32, tag="F2")
        F3 = feat.tile([128, N], F32, tag="F3")
        F4a = feat.tile([128, N], F32, tag="F4a")
        F4b = feat.tile([128, N], F32, tag="F4b")

        fins = [F0, F1, F2, F3]
        fouts = [[F1], [F2], [F3], [F4a, F4b]]

        # layer weights in SBUF
        wsb = []
        for li, ((c, o), (tA, tB, tbr)) in enumerate(zip(LAYERS, conv_w), start=1):
            Asb = const.tile([c, o], F32, tag=f"A{li}", name=f"A{li}sb")
            nc.sync.dma_start(Asb, tA)
            Bsb = const.tile([c, o], F32, tag=f"B{li}", name=f"B{li}sb")
            nc.sync.dma_start(Bsb, tB)
            brsb = const.tile([1, o], F32, tag=f"br{li}", name=f"br{li}sb")
            nc.sync.dma_start(brsb, tbr)
            wsb.append((Asb, Bsb, brsb))

        # ------------------- EdgeConv layers -------------------
        lctx = ctx.enter_context(ExitStack())
        lw = lctx.enter_context(tc.tile_pool(name="lw", bufs=1))
        luv = lctx.enter_context(tc.tile_pool(name="luv", bufs=3))
        lk = lctx.enter_context(tc.tile_pool(name="lk", bufs=3))
        lkd = lctx.enter_context(tc.tile_pool(name="lkd", bufs=4, space="DRAM"))
        psUV = lctx.enter_context(tc.tile_pool(name="psUV", bufs=2, space="PSUM"))
        psD = lctx.enter_context(tc.tile_pool(name="psD", bufs=2, space="PSUM"))
        psT = lctx.enter_context(tc.tile_pool(name="psT", bufs=2, space="PSUM"))
        for li, ((c, o), (Asb, Bsb, brsb)) in enumerate(zip(LAYERS, wsb), start=1):
            Fin = fins[li - 1]
            Fout = fouts[li - 1]
            PW = max(o, 128)  # padded fp16 row width (>=256B per gather row)
            UTd = dram.tile([N, PW], F16, tag=f"UT{li}", name=f"UT{li}d")

            if True:
                if True:
                    # prep: P2 = 2*Fin (ACT), PP = Fin^2 (ACT), negxx
                    P2 = lw.tile([c, N], F32, tag="p2", name="p2")
                    nc.scalar.activation(P2, Fin, Act.Copy, bias=0.0, scale=2.0)
                    PP = lw.tile([c, N], F32, tag="pp", name="pp")
                    nc.scalar.activation(PP, Fin, Act.Square)
                    negxx = lw.tile([1, N], F32, tag="negxx", name="negxx")
                    for ch in range(4):
                        xps = psUV.tile([1, 512], F32, tag="uv", name="xps")
                        nc.tensor.matmul(
                            xps, onescol[0:c, :], PP[:, ch * 512:(ch + 1) * 512],
                            start=True, stop=True,
                        )
                        nc.scalar.activation(
                            negxx[:, ch * 512:(ch + 1) * 512], xps, Act.Copy,
                            bias=0.0, scale=-1.0,
                        )

                    dds_q = []

                    def produce(t, nc=nc, lk=lk, psD=psD, P2=P2, Fin=Fin,
                                negxx=negxx, onesrow=onesrow):
                        dds = lk.tile([128, N], F32, tag="dds", name="dds",
                                      bufs=4)
                        for hh in range(2):
                            ddp = psD.tile([128, N // 2], F32, tag="dd",
                                           name="ddp")
                            for ch in range(2):
                                cs = slice(hh * 1024 + ch * 512,
                                           hh * 1024 + (ch + 1) * 512)
                                cp = slice(ch * 512, (ch + 1) * 512)
                                nc.tensor.matmul(
                                    ddp[:, cp],
                                    P2[:, t * 128:(t + 1) * 128],
                                    Fin[:, cs], start=True, stop=False,
                                )
                                nc.tensor.matmul(
                                    ddp[:, cp], onesrow, negxx[:, cs],
                                    start=False, stop=True,
                                )
                            nc.scalar.activation(
                                dds[:, hh * 1024:(hh + 1) * 1024], ddp,
                                Act.Copy)
                        return dds

                    for _pt in range(min(3, NT)):
                        dds_q.append(produce(_pt))

                    # U^T -> DRAM;  V^T -> SBUF
                    Vt = lw.tile([128, NT * o], F32, tag="vt", name="vt")
                    for t in range(NT):
                        sl = slice(t * 128, (t + 1) * 128)
                        ups = psUV.tile([128, o], F32, tag="uv", name="ups")
                        nc.tensor.matmul(ups, Fin[:, sl], Asb, start=True, stop=True)
                        usb = luv.tile([128, PW], F16, tag="usb", name="usb")
                        if PW > o:
                            nc.gpsimd.memset(usb[:, o:PW], 0)
                        nc.scalar.activation(usb[:, 0:o], ups, Act.Copy)
                        nc.sync.dma_start(UTd[sl, :], usb)
                        vps = psUV.tile([128, o], F32, tag="uv", name="vps")
                        nc.tensor.matmul(vps, Fin[:, sl], Bsb, start=True, stop=False)
                        nc.tensor.matmul(vps, onesrow, brsb, start=False, stop=True)
                        nc.scalar.activation(
                            Vt[:, t * o:(t + 1) * o], vps, Act.Copy
                        )

                # KNN + gather + max per point tile
                if True:
                    pend = []

                    def consume(nc=nc, lk=lk, psT=psT, Vt=Vt, Fout=Fout, o=o):
                        t, nbrv = pend.pop(0)
                        sl = slice(t * 128, (t + 1) * 128)
                        # max over k: fp16 pairwise-max tree (2x DVE mode)
                        for half in (16, 8, 4, 2, 1):
                            nc.vector.tensor_tensor(
                                nbrv[:, 0:half, 0:o],
                                nbrv[:, 0:half, 0:o],
                                nbrv[:, half:2 * half, 0:o],
                                Alu.max,
                            )
                        M = lk.tile([128, o], F32, tag="m", name="mtile")
                        nc.vector.tensor_tensor(
                            M, nbrv[:, 0, 0:o], Vt[:, t * o:(t + 1) * o],
                            Alu.add
                        )
                        act = lk.tile([128, o], F32, tag="act", name="acttile")
                        _leaky(nc, act, M)
                        for bi, Fo in enumerate(Fout):
                            bw = min(128, o - bi * 128)
                            tp = psT.tile([128, 128], F32, tag="tp", name="tp")
                            nc.tensor.transpose(
                                tp[0:bw, :], act[:, bi * 128:bi * 128 + bw],
                                ident
                            )
                            nc.scalar.activation(
                                Fo[0:bw, sl], tp[0:bw, :], Act.Copy)

                    for t in range(NT):
                        sl = slice(t * 128, (t + 1) * 128)
                        dds = dds_q.pop(0)
                        if t + 3 < NT:
                            dds_q.append(produce(t + 3))

                        gidx = lk.tile([128, K], U16, tag="gidx", name="gidx")
                        for r in range(4):
                            v8 = lk.tile([128, 8], F32, tag="v8", name="v8")
                            nc.vector.max(v8, dds)
                            nc.vector.max_index(gidx[:, r * 8:(r + 1) * 8], v8, dds)
                            if r < 3:
                                nc.vector.match_replace(dds, v8, dds, NEG_INF)

                        if t_dbg is not None and li == 1:
                            gidxf = lk.tile([128, K], F32, tag="gidxf",
                                            name="gidxf")
                            nc.vector.tensor_copy(gidxf, gidx)
                            nc.sync.dma_start(
                                t_dbg["dgidx1"][:, t * K:(t + 1) * K], gidxf
                            )
                        # SWDGE wrapped index layout: list[i] lives at
                        # storage[i % 16, i // 16]; we need
                        # list[k*128 + p] = gidx[p, k]  =>
                        # widx[q, 8k+u] = gidx[16u+q, k]
                        ld = lkd.tile([128, K], I16, tag="ld", name="ld")
                        nc.sync.dma_start(ld, gidx.bitcast(I16))
                        widx = lk.tile([128, 256], I16, tag="widx", name="widx")
                        ldw = ld.rearrange("(u q) k -> q k u", u=8)
                        nc.sync.dma_start(
                            widx[0:16, :].rearrange("q (k u) -> q k u", u=8),
                            ldw,
                        )
                        for g in range(1, 8):
                            nc.sync.dma_start(
                                widx[g * 16:(g + 1) * 16, :], widx[0:16, :]
                            )

                        nbr = lk.tile([128, K * PW], F16, tag="nbr", name="nbr")
                        nbrv = nbr.rearrange("p (k o) -> p k o", k=K)
                        for gc in range(4):
                            nc.gpsimd.dma_gather(
                                nbrv[:, gc * 8:(gc + 1) * 8, :],
                                UTd,
                                widx[:, gc * 64:(gc + 1) * 64],
                                1024,
                                1024,
                                PW,
                                queue_num=(t * 4 + gc) % 4,
                            )

                        pend.append((t, nbrv))
                        if len(pend) >= 2:
                            consume()
                    while pend:
                        consume()

        if t_dbg is not None:
            nc.sync.dma_start(t_dbg["dF1"], F1)
            nc.sync.dma_start(t_dbg["dF2"], F2)
            nc.sync.dma_start(t_dbg["dF3"], F3)
            nc.sync.dma_start(t_dbg["dF4a"], F4a)
            nc.sync.dma_start(t_dbg["dF4b"], F4b)

        lctx.close()

        # ------------------- global feature + heads -------------------
        with tc.tile_pool(name="fcw", bufs=1) as fcw, \
             tc.tile_pool(name="fcwk", bufs=2) as fcwk, \
             tc.tile_pool(name="psh", bufs=2, space="PSUM") as psH, \
             tc.tile_pool(name="psf", bufs=4, space="PSUM") as psF:
            A5sb = []
            for i, tA in enumerate(t_A5):
                p = 64 if i < 2 else 128
                a = fcw.tile([p, 1024], F32, tag=f"A5_{i}", name=f"A5sb{i}")
                nc.sync.dma_start(a, tA)
                A5sb.append(a)
            b5t = fcw.tile([128, 8], F32, tag="b5t")
            nc.sync.dma_start(b5t, t_b5.rearrange("(i p) -> p i", p=128))

            L1Asb = fcw.tile([128, 16 * 512], F32, tag="L1A", name="L1Asb")
            nc.sync.dma_start(
                L1Asb.rearrange("p (j o) -> p j o", j=16),
                t_L1A.rearrange("(j p) o -> p j o", p=128),
            )
            F1Asb = fcw.tile([128, 8 * 512], F32, tag="F1A", name="F1Asb")
            nc.sync.dma_start(
                F1Asb.rearrange("p (j o) -> p j o", j=8),
                t_F1A.rearrange("(j p) o -> p j o", p=128),
            )
            L2Asb = fcw.tile([128, 4 * 256], F32, tag="L2A", name="L2Asb")
            nc.sync.dma_start(
                L2Asb.rearrange("p (j o) -> p j o", j=4),
                t_L2A.rearrange("(j p) o -> p j o", p=128),
            )
            F2Asb = fcw.tile([128, 4 * 256], F32, tag="F2A", name="F2Asb")
            nc.sync.dma_start(
                F2Asb.rearrange("p (j o) -> p j o", j=4),
                t_F2A.rearrange("(j p) o -> p j o", p=128),
            )
            L3Asb = fcw.tile([128, 2 * 5], F32, tag="L3A", name="L3Asb")
            nc.sync.dma_start(
                L3Asb.rearrange("p (j o) -> p j o", j=2),
                t_L3A.rearrange("(j p) o -> p j o", p=128),
            )
            F3Asb = fcw.tile([128, 2 * 5], F32, tag="F3A", name="F3Asb")
            nc.sync.dma_start(
                F3Asb.rearrange("p (j o) -> p j o", j=2),
                t_F3A.rearrange("(j p) o -> p j o", p=128),
            )
            b6sb = fcw.tile([1, 512], F32, tag="b6")
            nc.sync.dma_start(b6sb, t_b6)
            b8sb = fcw.tile([1, 512], F32, tag="b8")
            nc.sync.dma_start(b8sb, t_b8)
            L2bsb = fcw.tile([1, 256], F32, tag="L2b")
            nc.sync.dma_start(L2bsb, t_L2b)
            F2bsb = fcw.tile([1, 256], F32, tag="F2b")
            nc.sync.dma_start(F2bsb, t_F2b)
            L3bsb = fcw.tile([1, 5], F32, tag="L3b")
            nc.sync.dma_start(L3bsb, t_L3b)
            F3bsb = fcw.tile([1, 5], F32, tag="F3b")
            nc.sync.dma_start(F3bsb, t_F3b)

            # h = LeakyReLU(A5 @ cat + b5) in [o, n] layout; pool max+sum
            pieces = [
                (A5sb[0], F1, 64),
                (A5sb[1], F2, 64),
                (A5sb[2], F3, 128),
                (A5sb[3], F4a, 128),
                (A5sb[4], F4b, 128),
            ]
            maxh = fcw.tile([128, 8], F32, tag="maxh")
            sumh = fcw.tile([128, 8], F32, tag="sumh")
            for ot in range(8):
                osl = slice(ot * 128, (ot + 1) * 128)
                cmax = fcwk.tile([128, 4], F32, tag="cmax", name="cmax")
                csum = fcwk.tile([128, 4], F32, tag="csum", name="csum")
                for nch in range(4):
                    nsl = slice(nch * 512, (nch + 1) * 512)
                    hps = psH.tile([128, 512], F32, tag="h", name="hps")
                    for i, (Ax, Fx, kk) in enumerate(pieces):
                        nc.tensor.matmul(
                            hps, Ax[:, osl], Fx[:, nsl],
                            start=(i == 0), stop=(i == len(pieces) - 1),
                        )
                    ht = fcwk.tile([128, 512], F32, tag="ht", name="ht")
                    # ht = h + b5, then leaky with sum accum (DVE)
                    nc.vector.tensor_scalar(
                        ht, hps, b5t[:, ot:ot + 1], None, Alu.add
                    )
                    hl = fcwk.tile([128, 512], F32, tag="hl", name="hl")
                    _leaky(nc, hl, ht, accum_out=csum[:, nch:nch + 1])
                    nc.vector.tensor_reduce(
                        cmax[:, nch:nch + 1], hl, Axis.X, Alu.max
                    )
                nc.vector.tensor_reduce(
                    maxh[:, ot:ot + 1], cmax, Axis.X, Alu.max
                )
                nc.vector.tensor_reduce(
                    sumh[:, ot:ot + 1], csum, Axis.X, Alu.add
                )

            if t_dbg is not None:
                nc.sync.dma_start(t_dbg["dmaxh"], maxh)
                nc.sync.dma_start(t_dbg["dsumh"], sumh)

            def fc(lhs_sb, nj, rhs_cols, bias_sb, width, out_cols, act_fn=True):
                """out[width] = (LeakyReLU?)(lhsT.T @ rhs + bias). Returns
                [128, ceil(width/128)] tile whose columns are 128-chunks."""
                nm = (width + 127) // 128
                res = fcwk.tile([128, max(nm, 1)], F32, tag=f"fc{width}_{nj}",
                                name="fcres")
                for m in range(nm):
                    mw = min(128, width - m * 128)
                    zps = psF.tile([128, 1], F32, tag="z", name="zps")
                    for j in range(nj):
                        nc.tensor.matmul(
                            zps[0:mw, :],
                            lhs_sb.rearrange("p (j o) -> p j o", j=nj)[
                                :, j, m * 128:m * 128 + mw
                            ],
                            rhs_cols[j],
                            start=(j == 0), stop=False,
                        )
                    nc.tensor.matmul(
                        zps[0:mw, :],
                        bias_sb[:, m * 128:m * 128 + mw],
                        onesrow[:, 0:1],
                        start=False, stop=True,
                    )
                    nc.scalar.activation(
                        res[0:mw, m:m + 1], zps[0:mw, :], Act.Copy
                    )
                if act_fn:
                    _leaky(nc, res, res)
                return res

            # x-branch (g): L1 (K=2048: max||sum), L2, L3
            g_rhs = [maxh[:, j:j + 1] for j in range(8)] + \
                    [sumh[:, j:j + 1] for j in range(8)]
            z1 = fc(L1Asb, 16, g_rhs, b6sb, 512, 4)
            z1_rhs = [z1[:, j:j + 1] for j in range(4)]
            z2 = fc(L2Asb, 4, z1_rhs, L2bsb, 256, 2)
            z2_rhs = [z2[:, j:j + 1] for j in range(2)]
            z3 = fc(L3Asb, 2, z2_rhs, L3bsb, 5, 1, act_fn=False)
            nc.sync.dma_start(t_go, z3[0:5, 0:1])

            # y-branch: F1 (K=1024: max only), F2, F3
            y_rhs = [maxh[:, j:j + 1] for j in range(8)]
            w1 = fc(F1Asb, 8, y_rhs, b8sb, 512, 4)
            w1_rhs = [w1[:, j:j + 1] for j in range(4)]
            w2 = fc(F2Asb, 4, w1_rhs, F2bsb, 256, 2)
            w2_rhs = [w2[:, j:j + 1] for j in range(2)]
            w3 = fc(F3Asb, 2, w2_rhs, F3bsb, 5, 1, act_fn=False)
            nc.sync.dma_start(t_yo, w3[0:5, 0:1])


# --------------------------------------------------------------------------
# host side
# --------------------------------------------------------------------------

_NC = None


def _get_nc():
    global _NC
    if _NC is None:
        _NC = build_module()
    return _NC


def _prep_weights(inp):
    f = lambda k: np.ascontiguousarray(np.asarray(inp[k], dtype=np.float32))
    d = {}

    for li, (c, o) in enumerate(LAYERS, start=1):
        W = f(f"W{li}")          # [o, 2c]
        s = f(f"s{li}")          # [o]
        b = f(f"b{li}")          # [o]
        Wn = W[:, :c]
        Wc = W[:, c:]
        d[f"A{li}"] = np.ascontiguousarray((s[:, None] * Wn).T)
        d[f"B{li}"] = np.ascontiguousarray((s[:, None] * (Wc - Wn)).T)
        d[f"br{li}"] = b[None, :].copy()

    A5 = np.ascontiguousarray((f("s5")[:, None] * f("W5")).T)   # [512, 1024]
    d["A51"] = A5[0:64].copy()
    d["A52"] = A5[64:128].copy()
    d["A53"] = A5[128:256].copy()
    d["A54a"] = A5[256:384].copy()
    d["A54b"] = A5[384:512].copy()
    d["b5v"] = f("b5")

    L1 = (f("s6")[:, None] * f("L1w")).T.copy()                 # [2048, 512]
    L1[1024:] /= float(N)
    d["L1A"] = np.ascontiguousarray(L1)
    d["b6r"] = f("b6")[None, :].copy()
    d["L2A"] = np.ascontiguousarray((f("s7")[:, None] * f("L2w")).T)
    d["L2br"] = (f("s7") * f("L2b") + f("b7"))[None, :].copy()
    d["L3A"] = np.ascontiguousarray(f("L3w").T)
    d["L3br"] = f("L3b")[None, :].copy()

    d["F1A"] = np.ascontiguousarray((f("s8")[:, None] * f("F1w")).T)
    d["b8r"] = f("b8")[None, :].copy()
    d["F2A"] = np.ascontiguousarray((f("s9")[:, None] * f("F2w")).T)
    d["F2br"] = (f("s9") * f("F2b") + f("b9"))[None, :].copy()
    d["F3A"] = np.ascontiguousarray(f("F3w").T)
    d["F3br"] = f("F3b")[None, :].copy()

    d["ident"] = np.eye(128, dtype=np.float32)
    d["onesrow"] = np.ones((1, 128), dtype=np.float32)
    d["onescol"] = np.ones((128, 1), dtype=np.float32)
    return d


def kernel(**inputs):
    x = np.asarray(inputs["x"], dtype=np.float32)   # [8, 3, N]
    B = x.shape[0]
    assert B == 8 and x.shape[1] == 3 and x.shape[2] == N

    shared = _prep_weights(inputs)
    in_maps = []
    for bidx in range(B):
        m = dict(shared)
        m["xb"] = np.ascontiguousarray(x[bidx])
        in_maps.append(m)

    nc = _get_nc()
    res = run_bass_kernel_spmd(nc, in_maps, core_ids=list(range(B)))
    g = np.stack([res.results[i]["go"].reshape(5) for i in range(B)])
    y = np.stack([res.results[i]["yo"].reshape(5) for i in range(B)])
    return (g.astype(np.float32), y.astype(np.float32))


if __name__ == "__main__":
    # smoke test with random data
    rng = np.random.default_rng(0)
    print("building module...")
    nc = _get_nc()
    print("built ok")
